# revision 11
# baseline (speedup 1.0000x reference)
"""APPNP (MLP + 10-step personalized propagation) on 8 TRN2 NeuronCores.

Strategy
--------
- Nodes are sharded across the 8 cores by destination row (12500 rows/core).
- The small MLP runs on the TensorEngine with x pre-transposed on the host.
- h lives between steps as an fp16 "quad table" in DRAM: 4 consecutive
  (permuted) node rows interleaved feature-major into one 256B table row.
  The table is exactly the AllGather output buffer - no rebuild step.
- Each propagation step, every core gathers the source-node quads for its
  edges with dma_gather (SWDGE, 4 queues) in an ELL slot layout: slot
  (p, s) holds one edge of one of partition p's rows.  Rows are dealt to
  (partition, group) by global degree rank so the per-group slot count D_g
  is tight across all cores (same NEFF for all cores).
- DVE does the multiply by the (val x quad-half mask) stream and a
  segmented reduce per row group; fp32 accumulation.
- The self-loop term alpha*h is folded in as one extra slot per row whose
  value is alpha (edge vals carry the (1-alpha) factor).
"""

import os
import sys
import types

sys.path.insert(0, "/opt/trn_rl_repo")

import numpy as np

N_NODES = 100000
N_EDGES = 1600000
IN_F, HID, OUT_F = 512, 128, 32
ALPHA = 0.01
K_STEPS = int(os.environ.get("APPNP_STEPS", "10"))
CORES = 8
R_CORE = N_NODES // CORES        # 12500 real rows per core
P = 128                          # partitions
NG = 100                         # row groups per partition (2 all-hole)
NG_REAL = 98                     # groups that carry real rows
SH = P * NG                      # 12800 shard rows incl. holes
UQ = NG // 4                     # 25 interleaved quads per partition
TAB_QUADS = CORES * P * UQ       # 25600 table rows of 128 fp16
ELEM = 128                       # fp16 elems per table row (256B)
MAX_CALL_SLOTS = 80              # slots per dma_gather call (per queue buf)
MLP_CHUNK = 256                  # columns per MLP chunk


def _install_trace_hook():
    """Register the NTFF profiling hook so run_bass_kernel_spmd(trace=True)
    returns exec_time_ns in this container."""
    import antenv
    if "antenv.axon_hooks" not in sys.modules:
        mod = types.ModuleType("antenv.axon_hooks")
        _hook = [None]
        mod.set_axon_ntff_profile_hook = lambda h: _hook.__setitem__(0, h)
        mod.get_axon_ntff_profile_hook = lambda: _hook[0]
        sys.modules["antenv.axon_hooks"] = mod
        antenv.axon_hooks = mod
        try:
            from trn_agent_boot.trn_boot import _ntff_profile_via_ctypes
            mod.set_axon_ntff_profile_hook(
                _ntff_profile_via_ctypes("/opt/axon/libaxon_pjrt.so"))
        except Exception as e:  # profiling optional
            print("ntff hook registration failed:", e)
    from concourse import bass_utils
    bass_utils.upload_artifacts = lambda tmpdir: tmpdir


# ----------------------------------------------------------------------------
# Host-side preprocessing
# ----------------------------------------------------------------------------

def preprocess(edge_row, edge_col, edge_val):
    """Build the ELL geometry + per-core index/value streams.

    Returns dict with geometry (shared by all cores -> one NEFF) and
    per-core numpy arrays.
    """
    er = np.asarray(edge_row, dtype=np.int64)
    ec = np.asarray(edge_col, dtype=np.int64)
    ev = np.asarray(edge_val, dtype=np.float32)

    deg = np.bincount(er, minlength=N_NODES)
    need = deg + 1                                   # +1 self-loop slot

    # Deal rows of each core to (partition, group) by descending slot need.
    p_of = np.empty(N_NODES, np.int32)
    g_of = np.empty(N_NODES, np.int32)
    # per (core, group): max need among its 128 rows
    Dg_per_core = np.zeros((CORES, NG), np.int32)
    for m in range(CORES):
        lo = m * R_CORE
        order = np.argsort(-need[lo:lo + R_CORE], kind="stable")  # local rows
        ranks = np.arange(R_CORE)
        p_loc = np.empty(R_CORE, np.int32)
        g_loc = np.empty(R_CORE, np.int32)
        p_loc[order] = (ranks % P).astype(np.int32)
        g_loc[order] = (ranks // P).astype(np.int32)
        p_of[lo:lo + R_CORE] = p_loc
        g_of[lo:lo + R_CORE] = g_loc
        sorted_need = need[lo:lo + R_CORE][order]
        for g in range(NG_REAL):
            a, b = g * P, min((g + 1) * P, R_CORE)
            if a < R_CORE:
                Dg_per_core[m, g] = sorted_need[a]   # max of the slab
    D = Dg_per_core.max(axis=0)                      # shared geometry
    D = np.maximum(D, 1)
    D[NG_REAL:] = 0                                  # hole-only groups
    S = np.concatenate([[0], np.cumsum(D)]).astype(np.int64)
    n_slots = int(S[NG])

    # pi position of every node: core*SH + 100*p + g (p-major within core)
    pi = (er // R_CORE) * 0  # placeholder to keep shape; computed per node:
    node = np.arange(N_NODES, dtype=np.int64)
    pi = (node // R_CORE) * SH + NG * p_of + g_of    # [N] int64
    # table quad index + intra-quad half for a node (interleaved layout):
    # core m, partition p, uquad u=g//4  ->  quad = m*P*UQ + p*UQ + u
    quad_of = (node // R_CORE) * (P * UQ) + p_of.astype(np.int64) * UQ + (g_of // 4)
    half_of = (g_of % 4).astype(np.int64)
    assert quad_of.max() < TAB_QUADS <= 32767

    # Per-edge slot assignment (edges sorted by destination row).
    order_e = np.argsort(er, kind="stable")
    er_s = er[order_e]
    ec_s = ec[order_e]
    ev_s = ev[order_e]
    row_start = np.searchsorted(er_s, node)
    within = np.arange(N_EDGES, dtype=np.int64) - row_start[er_s]
    core_e = er_s // R_CORE
    pe_ = p_of[er_s].astype(np.int64)
    ge_ = g_of[er_s].astype(np.int64)
    s_e = S[ge_] + within

    gq = np.zeros((CORES, P, n_slots), np.int16)
    vex = np.zeros((CORES, P, n_slots, 4), np.float16)
    gq[core_e, pe_, s_e] = quad_of[ec_s].astype(np.int16)
    vex[core_e, pe_, s_e, half_of[ec_s]] = ((1.0 - ALPHA) * ev_s).astype(np.float16)

    # self-loop slots (one per real row) at position S[g] + deg
    core_n = node // R_CORE
    s_self = S[g_of.astype(np.int64)] + deg
    gq[core_n, p_of, s_self] = quad_of.astype(np.int16)
    vex[core_n, p_of, s_self, half_of] = np.float16(ALPHA)

    # Call split: consecutive groups, <= MAX_CALL_SLOTS slots per call.
    calls = []  # (g0, g1, s0, s1)
    g0 = 0
    while g0 < NG_REAL:
        g1 = g0
        while g1 < NG_REAL and S[g1 + 1] - S[g0] <= MAX_CALL_SLOTS:
            g1 += 1
        calls.append((g0, g1, int(S[g0]), int(S[g1])))
        g0 = g1
    maxw = max(s1 - s0 for _, _, s0, s1 in calls)

    # Wrapped idx stream per call: position i=(s-s0)*128+p -> [i%16, i//16]
    idx_parts = []
    for (_, _, s0, s1) in calls:
        blk = gq[:, :, s0:s1]                        # [CORES, P, W]
        W = s1 - s0
        stream = np.transpose(blk, (0, 2, 1)).reshape(CORES, W * P)
        wrapped = stream.reshape(CORES, W * P // 16, 16).transpose(0, 2, 1)
        idx_parts.append(wrapped)                    # [CORES, 16, W*8]
    idx16 = np.concatenate(idx_parts, axis=2)        # [CORES, 16, n_slots*8]
    idx_full = np.tile(idx16, (1, 8, 1))             # [CORES, 128, n_slots*8]

    # node -> output position (for host-side unpermute of the result)
    out_pos = (node % R_CORE) * 0 + NG * p_of + g_of  # within-core [0, SH)

    return dict(
        D=D, S=S, n_slots=n_slots, calls=calls, maxw=maxw,
        idx=np.ascontiguousarray(idx_full),
        vex=np.ascontiguousarray(vex.reshape(CORES, P, n_slots * 4)),
        p_of=p_of, g_of=g_of, out_pos=out_pos,
    )


# ----------------------------------------------------------------------------
# Bass graph
# ----------------------------------------------------------------------------

def build(geom):
    import concourse.bass as bass
    import concourse.bacc as bacc
    import concourse.mybir as mybir
    from concourse.masks import make_identity

    D, S, calls = geom["D"], geom["S"], geom["calls"]
    n_slots, maxw = geom["n_slots"], geom["maxw"]
    DMAX = int(D.max())
    n_calls = len(calls)
    nchunks = SH // MLP_CHUNK                        # 50
    f32, f16, i16 = mybir.dt.float32, mybir.dt.float16, mybir.dt.int16

    nc = bacc.Bacc(num_swdge_queues=4, detect_race_conditions=False, dynamic_dma_scratch_size=65536)

    xt_ext = nc.declare_dram_parameter("xt", [IN_F, SH], f32, isOutput=False)
    gidx_ext = nc.declare_dram_parameter("gidx", [P, n_slots * 8], i16, isOutput=False)
    vex_ext = nc.declare_dram_parameter("vex", [P, n_slots * 4], f16, isOutput=False)
    w1_ext = nc.declare_dram_parameter("w1", [4, P, HID], f32, isOutput=False)
    w2_ext = nc.declare_dram_parameter("w2", [HID, OUT_F], f32, isOutput=False)
    b1_ext = nc.declare_dram_parameter("b1", [HID, 1], f32, isOutput=False)
    b2_ext = nc.declare_dram_parameter("b2", [OUT_F, 1], f32, isOutput=False)
    out_ext = nc.declare_dram_parameter("out", [P, UQ, OUT_F, 4], f32, isOutput=True)

    shard_d = nc.dram_tensor("shard_d", [P * UQ, OUT_F, 4], f16)
    table_d = nc.dram_tensor("table_d", [TAB_QUADS, ELEM], f16, addr_space="Shared")

    # number of gather calls issued on queue q up to and including global
    # call index c (cumulative over steps)
    def q_of(c):
        return c % 4

    from contextlib import ExitStack
    es = ExitStack()
    with es:
        block = es.enter_context(nc.Block())
        sem = lambda n: es.enter_context(nc.semaphore(n))
        sb = lambda n, shp, dt: es.enter_context(nc.sbuf_tensor(n, shp, dt))
        ps = lambda n, shp, dt: es.enter_context(nc.psum_tensor(n, shp, dt))
        ld_sem = sem("ld_sem"); xs_sem = sem("xs_sem")
        pe1_sem = sem("pe1_sem"); pex_sem = sem("pex_sem")
        pe2_sem = sem("pe2_sem"); pet_sem = sem("pet_sem")
        act1_sem = sem("act1_sem"); act2_sem = sem("act2_sem")
        acte_sem = sem("acte_sem"); cast_sem = sem("cast_sem")
        ag_sem = sem("ag_sem")
        q0_sem = sem("q0_sem"); q1_sem = sem("q1_sem")
        q2_sem = sem("q2_sem"); q3_sem = sem("q3_sem")
        dve_sem = sem("dve_sem"); init_sem = sem("init_sem")
        idx_sb = sb("idx_sb", [P, n_slots * 8], i16)
        vex_sb = sb("vex_sb", [P, n_slots, 4], f16)
        g0_sb = sb("g0_sb", [P, maxw, ELEM], f16)
        g1_sb = sb("g1_sb", [P, maxw, ELEM], f16)
        g2_sb = sb("g2_sb", [P, maxw, ELEM], f16)
        g3_sb = sb("g3_sb", [P, maxw, ELEM], f16)
        m_sb = sb("m_sb", [P, OUT_F, DMAX, 4], f16)
        hnew = sb("hnew", [P, UQ, OUT_F, 4], f32)
        w1_sb = sb("w1_sb", [P, 4, HID], f32)
        w2_sb = sb("w2_sb", [HID, OUT_F], f32)
        b1_sb = sb("b1_sb", [HID, 1], f32)
        b2_sb = sb("b2_sb", [OUT_F, 1], f32)
        xt0_sb = sb("xt0_sb", [P, 4, MLP_CHUNK], f32)
        xt1_sb = sb("xt1_sb", [P, 4, MLP_CHUNK], f32)
        h1_sb = sb("h1_sb", [P, MLP_CHUNK], f32)
        h2_sb = sb("h2_sb", [OUT_F, MLP_CHUNK], f32)
        ident = sb("ident", [OUT_F, OUT_F], f32)
        ph1 = ps("ph1", [P, MLP_CHUNK], f32)
        ph2 = ps("ph2", [OUT_F, MLP_CHUNK], f32)
        pt0 = ps("pt0", [P, OUT_F], f32)
        pt1 = ps("pt1", [P, OUT_F], f32)

        xts = [xt0_sb, xt1_sb]
        pts = [pt0, pt1]
        gbufs = [g0_sb, g1_sb, g2_sb, g3_sb]
        qsems = [q0_sem, q1_sem, q2_sem, q3_sem]

        @block.sync
        def _(sp):
            sp.dma_start(out=idx_sb[:, :], in_=gidx_ext[:, :]).then_inc(ld_sem, 16)
            sp.dma_start(out=vex_sb[:, :, :], in_=vex_ext[:, :]).then_inc(ld_sem, 16)
            # w1 [4,128,HID] -> [128,4,HID]
            sp.dma_start(out=w1_sb[:, :, :],
                         in_=w1_ext[:, :, :].transpose([1, 0, 2])).then_inc(ld_sem, 16)
            sp.dma_start(out=w2_sb[:, :], in_=w2_ext[:, :]).then_inc(ld_sem, 16)
            sp.dma_start(out=b1_sb[:, :], in_=b1_ext[:, :]).then_inc(ld_sem, 16)
            sp.dma_start(out=b2_sb[:, :], in_=b2_ext[:, :]).then_inc(ld_sem, 16)
            for cc in range(nchunks):
                if cc >= 2:
                    sp.wait_ge(pe1_sem, cc - 1)
                lo = cc * MLP_CHUNK
                sp.dma_start(
                    out=xts[cc % 2][:, :, :],
                    in_=xt_ext[:, lo:lo + MLP_CHUNK].rearrange(
                        "(k p) c -> p k c", k=4),
                ).then_inc(xs_sem, 16)

        @block.tensor
        def _(pe):
            pe.wait_ge(ld_sem, 96)
            pe.wait_ge(init_sem, 1)  # identity ready
            for cc in range(nchunks):
                pe.wait_ge(xs_sem, 16 * (cc + 1))
                if cc >= 1:
                    pe.wait_ge(act1_sem, cc)      # ph1 free again
                for k in range(4):
                    mm = pe.matmul(out=ph1[:, :], lhsT=w1_sb[:, k, :],
                                   rhs=xts[cc % 2][:, k, :],
                                   start=(k == 0), stop=(k == 3))
                    if k == 3:
                        mm.then_inc(pe1_sem, 1)
                pe.wait_ge(act1_sem, cc + 1)      # h1_sb written
                if cc >= 1:
                    pe.wait_ge(act2_sem, cc)      # ph2 free
                pe.matmul(out=ph2[:, :], lhsT=w2_sb[:, :], rhs=h1_sb[:, :],
                          start=True, stop=True).then_inc(pe2_sem, 1)
                pe.wait_ge(act2_sem, cc + 1)      # h2_sb written
                for j in range(2):
                    t = 2 * cc + j
                    if t >= 2:
                        pe.wait_ge(acte_sem, t - 1)  # pt[t%2] evicted
                    pe.transpose(out=pts[t % 2][:, :],
                                 in_=h2_sb[:, j * P:(j + 1) * P],
                                 identity=ident[:, :]).then_inc(pet_sem, 1)

        @block.scalar
        def _(act):
            act.wait_ge(init_sem, 1)
            for cc in range(nchunks):
                act.wait_ge(pe1_sem, cc + 1)
                act.activation(out=h1_sb[:, :], in_=ph1[:, :],
                               func=mybir_relu(), bias=b1_sb[:, 0:1],
                               ).then_inc(act1_sem, 1)
                act.wait_ge(pe2_sem, cc + 1)
                act.activation(out=h2_sb[:, :], in_=ph2[:, :],
                               func=mybir_relu(), bias=b2_sb[:, 0:1],
                               ).then_inc(act2_sem, 1)
                for j in range(2):
                    t = 2 * cc + j
                    g = t            # global row-group index 0..99
                    act.wait_ge(pet_sem, t + 1)
                    act.copy(out=hnew[:, g // 4, :, g % 4],
                             in_=pts[t % 2][:, :]).then_inc(acte_sem, 1)

        @block.gpsimd
        def _(gp):
            gp.memset(hnew[:, :, :, :], 0.0)
            make_identity(nc, ident[:, :])
            gp.memset(m_sb[:, 0:1, 0:1, :], 0.0).then_inc(init_sem, 1)
            gp.wait_ge(ld_sem, 96)
            # prologue: wait MLP done, publish h0
            gp.wait_ge(acte_sem, 2 * nchunks)
            gp.dma_start(out=shard_d[:, :, :],
                         in_=hnew[:, :, :, :]).then_inc(cast_sem, 16)
            gp.wait_ge(cast_sem, 16)
            gp.collective_compute(
                "AllGather", mybir_bypass(),
                replica_groups=[list(range(CORES))],
                ins=[shard_d.ap().opt()],
                outs=[table_d.ap().opt()],
            ).then_inc(ag_sem, 1)

            groups_done = 0          # cumulative reduce count (= dve_sem)
            q_used = [0, 0, 0, 0]
            call_groups = []         # groups completed through call c
            for k in range(K_STEPS):
                for ci, (gA, gB, s0, s1) in enumerate(calls):
                    c = k * n_calls + ci
                    q = q_of(c)
                    gp.wait_ge(ag_sem, k + 1)
                    if c >= 4:
                        gp.wait_ge(dve_sem, call_groups[c - 4])
                    W = s1 - s0
                    gp.dma_gather(
                        out_ap=gbufs[q][:, 0:W, :],
                        in_ap=table_d[:, :],
                        idxs_ap=idx_sb[:, s0 * 8:s1 * 8],
                        num_idxs=W * P,
                        num_idxs_reg=W * P,
                        elem_size=ELEM,
                        single_packet=False,
                        queue_num=q,
                    ).then_inc(qsems[q], 16)
                    q_used[q] += 1
                    groups_done += (gB - gA)
                    call_groups.append(groups_done)
                # end of step: publish h
                gp.wait_ge(dve_sem, (k + 1) * NG_REAL)
                if k < K_STEPS - 1:
                    gp.dma_start(
                        out=shard_d[:, :, :],
                        in_=hnew[:, :, :, :]).then_inc(cast_sem, 16)
                    gp.wait_ge(cast_sem, 16 * (k + 2))
                    gp.collective_compute(
                        "AllGather", mybir_bypass(),
                        replica_groups=[list(range(CORES))],
                        ins=[shard_d.ap().opt()],
                        outs=[table_d.ap().opt()],
                    ).then_inc(ag_sem, 1)

        @block.vector
        def _(dve):
            import concourse.mybir as mybir
            q_count = [0, 0, 0, 0]
            for k in range(K_STEPS):
                dve.wait_ge(cast_sem, 16 * (k + 1))  # hnew free to overwrite
                for ci, (gA, gB, s0, s1) in enumerate(calls):
                    c = k * n_calls + ci
                    q = q_of(c)
                    q_count[q] += 1
                    dve.wait_ge(qsems[q], 16 * q_count[q])
                    for g in range(gA, gB):
                        Dg = int(D[g])
                        off = int(S[g]) - s0
                        gv = gbufs[q][:, off:off + Dg, :].rearrange(
                            "p d (f j) -> p f d j", j=4)
                        vv = vex_sb[:, int(S[g]):int(S[g]) + Dg, :].unsqueeze(
                            1).broadcast_to([P, OUT_F, Dg, 4])
                        dve.tensor_tensor(out=m_sb[:, :, 0:Dg, :], in0=gv,
                                          in1=vv, op=mybir.AluOpType.mult)
                        dve.tensor_reduce(
                            out=hnew[:, g // 4, :, g % 4],
                            in_=m_sb[:, :, 0:Dg, :],
                            axis=mybir.AxisListType.XY,
                            op=mybir.AluOpType.add,
                        ).then_inc(dve_sem, 1)

        @block.scalar
        def _(act):
            # final output DMA (HWDGE on scalar engine)
            act.wait_ge(dve_sem, K_STEPS * NG_REAL)
            act.dma_start(out=out_ext[:, :, :, :],
                          in_=hnew[:, :, :, :]).then_inc(ld_sem, 16)
            act.wait_ge(ld_sem, 112)

    nc.compile()
    return nc


def mybir_relu():
    import concourse.mybir as mybir
    return mybir.ActivationFunctionType.Relu


def mybir_bypass():
    import concourse.mybir as mybir
    return mybir.AluOpType.bypass


# ----------------------------------------------------------------------------
# Entry point
# ----------------------------------------------------------------------------

_NC_CACHE = {}
LAST_EXEC_NS = None


def kernel(x, edge_row, edge_col, edge_val, W1, b1, W2, b2):
    global LAST_EXEC_NS
    trace = os.environ.get("APPNP_TRACE", "0") == "1"
    if trace:
        _install_trace_hook()

    from concourse import bass_utils

    geom = preprocess(edge_row, edge_col, edge_val)

    key = (geom["n_slots"], tuple(geom["D"].tolist()))
    if key not in _NC_CACHE:
        _NC_CACHE[key] = build(geom)
    nc = _NC_CACHE[key]

    x = np.asarray(x, dtype=np.float32)
    W1 = np.asarray(W1, dtype=np.float32)
    W2 = np.asarray(W2, dtype=np.float32)
    b1 = np.asarray(b1, dtype=np.float32).reshape(HID, 1)
    b2 = np.asarray(b2, dtype=np.float32).reshape(OUT_F, 1)
    w1 = np.ascontiguousarray(W1.reshape(4, P, HID))

    p_of, g_of = geom["p_of"], geom["g_of"]
    in_maps = []
    for m in range(CORES):
        lo = m * R_CORE
        xt = np.zeros((IN_F, SH), np.float32)
        colpos = P * g_of[lo:lo + R_CORE] + p_of[lo:lo + R_CORE]
        xt[:, colpos] = x[lo:lo + R_CORE].T
        in_maps.append({
            "xt": xt,
            "gidx": geom["idx"][m],
            "vex": geom["vex"][m],
            "w1": w1, "w2": W2, "b1": b1, "b2": b2,
        })

    res = bass_utils.run_bass_kernel_spmd(
        nc, in_maps, core_ids=list(range(CORES)), trace=trace)
    LAST_EXEC_NS = res.exec_time_ns

    out = np.empty((N_NODES, OUT_F), np.float32)
    for m in range(CORES):
        o = res.results[m]["out"]            # [P, UQ, OUT_F, 4]
        o = np.transpose(o, (0, 1, 3, 2)).reshape(SH, OUT_F)  # pos p*NG+g
        lo = m * R_CORE
        pos = NG * p_of[lo:lo + R_CORE] + g_of[lo:lo + R_CORE]
        out[lo:lo + R_CORE] = o[pos]
    return out
